# revision 77
# baseline (speedup 1.0000x reference)
"""Multi-head causal self-attention on 8 Trainium2 NeuronCores.

Problem: x[2, 2048, 1024] @ w_attn[1024, 3072] (+b) -> split q,k,v (16 heads,
head_size 64) -> causal softmax attention -> out [2, 2048, 1024].

Sharding: 32 (batch, head) pairs across 8 cores -> each core handles one batch
and 4 consecutive heads (batch = core // 4, heads = (core % 4) * 4 + [0..3]).
Each core runs a fused QKV-projection + attention kernel on its slice; the
host assembles the full output. No collectives needed.

Per-core device kernel (the ACT exp stream is the co-bottleneck with the PE,
so the projection is dissolved into filler units dribbled through attention
instead of running as an ACT-idle prefix phase):
  - qkT layout is per HEAD PAIR: chunks [q01 | k01 | q23 | k23], each
    [128, T] bf16, so q_h and k_h share partition offset (h%2)*64 (w_qk
    columns host-reordered to match). Projection happens in 16 units
    U(n, qtr): 8 kc-stationary fp32r matmuls into a
    [128, 512] PSUM bank + one DVE bias-add (psum f32 -> bf16). V' is
    produced by 16 units V(jc) (8 matmuls + one DVE bias+pack to bf16 with
    a ones column). All 32 units live in one FIFO filler queue, popped by
    a baseline per-pair drain plus demand-driven flushes (QK of (h, gi)
    forces U(h, <=gi); PV of a pair forces its two V units). Head 0's
    attention starts after just U(0,0) (~2 us of PE), so the exp stream
    starts ~8 us in and the ACT engine never sits behind a projection
    phase.
  - Attention per (h, gi): S^T pair tiles [128, 1024] via kT-stationary
    bf16 matmuls, causally trimmed on the moving (query) dim; odd-slot
    diagonal chunks are column-shifted so each pair's live region is
    contiguous and ONE left-trimmed exp window covers it exactly -> pt
    bf16; tril mask on diagonal 128x128 sub-blocks (Pool). PV reoriented:
    out[i_chunk, d] accumulates in one PSUM bank ([128, 4x65] regions,
    one accumulation group per (h, gi): first write per address
    initializes, rest accumulate) with P^T blocks stationary and
    V'[128, 65] bf16 moving - only causal (ic >= cj) blocks stream,
    65 cycles each. No transpose matmuls anywhere.
  - Tail: DVE reciprocal of the ones-column sums, Pool per-partition
    scalar multiply straight out of PSUM into the output tiles, DMA per
    completed 4-chunk row block.

PSUM: psS 2x[128,1024] (4 banks) + po 2x[128,512] (2 banks) + filler
2x[128,512] (2 banks) = 8 banks exactly. The psS banks are DVE-zeroed once
up front so trimmed QK pairs leave only finite values in any never-written
columns an exp window reads. PV deferral depth must stay <= npairs(gi)-1
per group (deeper lets accumulation groups on the 2 rotating po banks
overlap and pt tiles outlive their buffer rotation).
"""

from collections import deque

import numpy as np

import concourse.bacc as bacc
import concourse.bass as bass
import concourse.tile as tile
from concourse import mybir

AF = mybir.ActivationFunctionType
F32 = mybir.dt.float32
F32R = mybir.dt.float32r
BF16 = mybir.dt.bfloat16

B, T, C = 2, 2048, 1024
H, D = 16, 64
HPC = 4                 # heads per core
NCORES = 8
KC = C // 128           # 8 contraction chunks
TC = T // 128           # 16 key/time chunks of 128
TG = T // 512           # 4 query groups of 512
NQK = 2 * HPC * D       # 512 q+k columns per core
NV = HPC * D            # 256 v columns per core
DP = D + 1              # v columns + ones column
SCALE = D ** -0.5
DRAIN = (4, 6, 4, 4)    # eager filler items per pair, by query group
PVDEPTH = (1, 3, 5, 7)  # deferred-PV queue depth, by query group


def build_nc(reps=1):
    """reps > 1 repeats the whole compute serially inside one NEFF -
    used only for differential on-device timing (tunnel overhead cancels)."""
    nc = bacc.Bacc("TRN2", target_bir_lowering=False, debug=False,
                   num_devices=NCORES)

    xT = nc.dram_tensor("xT", [C, T], BF16, kind="ExternalInput")
    # weights host-pre-shuffled to SBUF layout so their DMAs are contiguous
    # per partition (strided gathers would run at ~half DMA rate):
    # w_qk[p, n, kc, c] with n-chunk cols [q01|k01|q23|k23]; w_v[p, kc, c]
    w_qk = nc.dram_tensor("w_qk", [128, HPC, KC, 128], BF16,
                          kind="ExternalInput")
    w_v = nc.dram_tensor("w_v", [128, KC * NV], BF16, kind="ExternalInput")
    b_qk = nc.dram_tensor("b_qk", [NQK, 1], F32, kind="ExternalInput")
    b_v = nc.dram_tensor("b_v", [1, NV], F32, kind="ExternalInput")
    tri = nc.dram_tensor("tri", [128, 128], BF16, kind="ExternalInput")
    out = nc.dram_tensor("out", [T, NV], F32, kind="ExternalOutput")

    with tile.TileContext(nc) as tc:
      for rep in range(reps):
        with (
            tc.tile_pool(name=f"const{rep}", bufs=1) as cpool,
            tc.tile_pool(name=f"xw{rep}", bufs=1) as xw,
            tc.tile_pool(name=f"qkv{rep}", bufs=1) as qkv,
            tc.tile_pool(name=f"outp{rep}", bufs=1) as outp,
            tc.tile_pool(name=f"work{rep}", bufs=20) as work,
        ):
            # ---------- ACT exp-table warmup (hide the first-use load) ------
            warm = work.tile([128, 1], F32, tag="warm")
            nc.vector.memset(warm[:], 0.0)
            nc.scalar.activation(warm[:], warm[:], AF.Exp)

            # ---------- DMAs, in consumption order on the sync queue --------
            # U(0,0) runs first (needs wqk h0 + x quarter 0); V units need
            # wv/bv/tri by ~8us; wqk h1..h3 before x quarters 1..3 (their
            # units run later than (gi0, h1..h3)'s QK).
            # HWDGE issues ~1 DMACopy / 625ns, so coarse copies matter: one
            # strided copy per weight n-chunk and per x quarter. Quarter 0
            # stays per-kc so U(0,0)'s kc stream is paced by chunk arrival.
            wqk_sb = [xw.tile([128, KC, 128], BF16, tag=f"wqk{n}",
                              name=f"wqk{n}") for n in range(HPC)]
            xT_p = xT[:].rearrange("(kc p) t -> p kc t", p=128)
            # quarter 0 in four 2-chunk copies: few enough that the HWDGE
            # issue rate (~625ns/copy) isn't the startup bottleneck, small
            # enough that U(0/1,0)'s kc stream starts after the first copy
            xts0 = [xw.tile([128, 2, 512], BF16, tag=f"xt{kk}_0",
                            name=f"xt{kk}_0") for kk in range(4)]
            nc.sync.dma_start(wqk_sb[0][:], w_qk[:, 0])
            nc.sync.dma_start(wqk_sb[1][:], w_qk[:, 1])
            for kk in range(4):
                nc.sync.dma_start(
                    xts0[kk][:], xT_p[:, 2 * kk:2 * kk + 2, 0:512]
                )
            bqk_sb = cpool.tile([128, HPC, 1], F32)
            nc.sync.dma_start(
                bqk_sb[:], b_qk[:].rearrange("(c p) one -> p c one", p=128)
            )
            tri_sb = cpool.tile([128, 128], BF16)
            nc.sync.dma_start(tri_sb[:], tri[:])
            xq = [None] + [xw.tile([128, KC, 512], BF16, tag=f"xq{q}",
                                   name=f"xq{q}") for q in range(1, TG)]
            wv_sb = xw.tile([128, KC, NV], BF16)
            nc.sync.dma_start(
                wv_sb[:], w_v[:].rearrange("p (kc c) -> p kc c", kc=KC)
            )
            bv_sb = cpool.tile([128, NV], F32)
            nc.sync.dma_start(bv_sb[:], b_v[:].to_broadcast([128, NV]))
            for n in (2, 3):
                nc.sync.dma_start(wqk_sb[n][:], w_qk[:, n])
            for q in range(1, TG):
                nc.sync.dma_start(
                    xq[q][:], xT_p[:, :, q * 512:(q + 1) * 512]
                )

            def xslice(kc, q):
                return (xts0[kc // 2][:, kc % 2, :] if q == 0
                        else xq[q][:, kc, :])

            # ---------- persistent sbuf tiles ----------
            # n-chunk layout: [q01 | k01 | q23 | k23], so q_h and k_h sit at
            # the same partition offset (h%2)*64 of chunks (h//2)*2(+1) and
            # matmul operands share a base partition.
            qkts = [qkv.tile([128, T], BF16, tag=f"qk{n}", name=f"qk{n}")
                    for n in range(HPC)]
            # one tile per 512-row output block -> one strided DMA per block
            outb = [outp.tile([128, 4, NV], F32, tag=f"o{g}", name=f"o{g}")
                    for g in range(TG)]
            out_r = out[:].rearrange("(g ic p) c -> g p ic c", p=128, ic=4)

            with (
                tc.tile_pool(name=f"psS{rep}", bufs=2, space="PSUM") as psS,
                tc.tile_pool(name=f"psout{rep}", bufs=2, space="PSUM") as psout,
                tc.tile_pool(name=f"fps{rep}", bufs=2, space="PSUM") as fps,
            ):
                vs = [None] * TC
                v_done = [[False, False] for _ in range(TC)]  # per head-pair
                u_done = [[False] * TG for _ in range(HPC)]
                filler = deque()   # FIFO of (projection | v) unit items
                pending_pv = deque()  # (emit_fn, h, (cj, cj')) deferred pairs

                # p-state warm-up: ~4us of tiny matmuls under the initial DMA
                # wait so the first real matmuls run at full clock
                wt = work.tile([128, 16], F32, tag="wt")
                nc.vector.memset(wt[:], 0.0)
                pw = fps.tile([128, 512], F32, tag="f", name="pwarm")
                for i in range(220):
                    nc.tensor.matmul(pw[0:16, 0:4], wt[:, 0:16], wt[:, 0:4],
                                     start=True, stop=True)
                # zero both S-psum buffers once (DVE, while it idles under
                # the DMA wait) so trimmed QK pairs leave only finite values
                # in the dead columns the wide exp windows read
                for _ in range(2):
                    z = psS.tile([128, 1024], F32, tag="psS", name="zinit")
                    nc.vector.memset(z[:], 0.0)

                def queue_u2(na, nb, qtr, fini_act=False):
                    """qkts[na|nb][:, qtr] = (w_qk^T x^T-quarter) + bias for a
                    q/k chunk pair, kc-interleaved so both trail the x-chunk
                    DMA stream by one chunk instead of running back-to-back;
                    8+8 PE matmuls into two PSUM banks + 2 DVE bias-adds."""
                    pss = {n: fps.tile([128, 512], F32, tag="f",
                                       name=f"u{n}_{qtr}") for n in (na, nb)}

                    def mk_mm(n, kc):
                        def emit():
                            nc.tensor.matmul(
                                pss[n][:],
                                wqk_sb[n][:, kc, :],
                                xslice(kc, qtr),
                                start=(kc == 0),
                                stop=(kc == KC - 1),
                            )
                        return emit

                    def mk_fini(n):
                        def fini():
                            dst = qkts[n][:, qtr * 512:(qtr + 1) * 512]
                            if fini_act:
                                # startup-critical: the idle ACT engine
                                # finishes the first q/k tiles while DVE
                                # would still be queuing
                                nc.scalar.activation(
                                    dst, pss[n][:], AF.Identity,
                                    bias=bqk_sb[:, n, :], scale=1.0,
                                )
                            else:
                                nc.vector.tensor_scalar_add(
                                    dst, pss[n][:], bqk_sb[:, n, :],
                                )
                            u_done[n][qtr] = True
                        return fini

                    for kc in range(KC):
                        filler.append(mk_mm(na, kc))
                        filler.append(mk_mm(nb, kc))
                    filler.append(mk_fini(na))
                    filler.append(mk_fini(nb))

                def queue_v(jc, hp):
                    """vs[jc] head-pair hp = x^T[:, jc]^T @ w_v (+ bias, ones
                    column), packed bf16. Split per head pair so the h23
                    halves' deadlines land in the late (h2/h3) sweeps."""
                    if vs[jc] is None:
                        vs[jc] = qkv.tile([128, HPC, DP], BF16, tag=f"v{jc}",
                                          name=f"v{jc}")
                    vt = vs[jc]
                    ps = fps.tile([128, 512], F32, tag="f", name=f"v{jc}_{hp}")
                    cols = slice(hp * 128, (hp + 1) * 128)

                    def mk_mm(kc):
                        def emit():
                            xh = xslice(kc, jc // 4)
                            col = (jc % 4) * 128
                            nc.tensor.matmul(
                                ps[:, 0:128],
                                xh[:, col:col + 128],
                                wv_sb[:, kc, cols],
                                start=(kc == 0),
                                stop=(kc == KC - 1),
                            )
                        return emit

                    def fini():
                        # bias + pack 2 heads' v columns (bf16) in one op
                        nc.vector.tensor_add(
                            vt[:, 2 * hp:2 * hp + 2, 0:D], ps[:, 0:128],
                            bv_sb[:, cols],
                        )
                        nc.vector.memset(vt[:, 2 * hp:2 * hp + 2, D:DP], 1.0)
                        v_done[jc][hp] = True

                    for kc in range(KC):
                        filler.append(mk_mm(kc))
                    filler.append(fini)

                def drain_filler(n):
                    for _ in range(n):
                        if filler:
                            filler.popleft()()

                def flush_until(cond):
                    while filler and not cond():
                        filler.popleft()()

                def flush_filler():
                    while filler:
                        filler.popleft()()

                def flush_pv(depth=0):
                    """Emit deferred PV blocks until at most `depth` remain.
                    A deeper queue at the exp-heavy late groups lets the PE
                    run further ahead of the ACT stream."""
                    while len(pending_pv) > depth:
                        fn, hp, (c1, c2) = pending_pv.popleft()
                        flush_until(lambda: v_done[c1][hp] and v_done[c2][hp])
                        fn()

                # enqueue every unit up front, in deadline order. gi g needs
                # U(*, g) and V(4g..4g+3); diagonal-first pair order means
                # V(4g+2), V(4g+3) are consumed before V(4g), V(4g+1).
                # U23 sits before V(4g)/V(4g+1) so h0's second-pair PV pull
                # drags it through while ACT still has h0/h1 exp backlog,
                # instead of it bursting right before h2's first QK
                for g in range(TG):
                    queue_u2(0, 1, g, fini_act=(g == 0))
                    for jc in (4 * g + 2, 4 * g + 3, 4 * g, 4 * g + 1):
                        queue_v(jc, 0)
                    queue_u2(2, 3, g)
                    for jc in (4 * g + 2, 4 * g + 3, 4 * g, 4 * g + 1):
                        queue_v(jc, 1)

                def emit_attn_main(h, gi):
                    """QK -> exp -> (tril mask) -> PV, software-pipelined:
                    each pair's PV is emitted after the NEXT pair's QK so the
                    PE FIFO never heads-of-line-blocks on an exp. Diagonal
                    pairs first. Returns tail state."""
                    qn, kn = (h // 2) * 2, (h // 2) * 2 + 1
                    flush_until(lambda: all(u_done[n][q] for n in (qn, kn)
                                            for q in range(gi + 1)))
                    # cap cross-segment PV carryover: with depth[gi] <=
                    # npairs(gi), pendings at entry are from one segment
                    # back only, so po bank groups (2 rotating banks) never
                    # overlap
                    flush_pv(depth=2)
                    po_ = (h % 2) * D
                    qT = qkts[qn][po_:po_ + D, :]
                    kT = qkts[kn][po_:po_ + D, :]
                    qs = qT[:, gi * 512:(gi + 1) * 512]
                    ncj = gi * 4 + 4  # causal: j-chunks 0..gi*4+3
                    npair = ncj // 2
                    pair_order = list(range(npair - 1, -1, -1))  # diag first
                    # one PSUM bank holds all 4 query-chunk accumulators
                    # ([128, 65] regions at 128-col offsets) under a single
                    # accumulation group per (h, gi).
                    po = psout.tile([128, 512], F32, tag="po",
                                    name=f"po{h}_{gi}")
                    nblk = sum(1 for p in range(npair)
                               for cj in (2 * p, 2 * p + 1)
                               for icl in range(4) if 4 * gi + icl >= cj)
                    state = {"emitted": 0, "nblk": nblk}
                    for p in pair_order:
                        pair = (2 * p, 2 * p + 1)
                        pss = psS.tile([128, 1024], F32, tag="psS",
                                       name=f"pss{h}_{gi}_{p}")
                        # diagonal chunks in the odd slot are SHIFTED left so
                        # their live columns start at 512: each pair's live
                        # region is contiguous and one exp window covers it
                        # exactly (the psS banks were zeroed once up front,
                        # so any never-written column exps to a finite 1.0
                        # that PV never streams).
                        offs, shifts = [], []
                        for m, cj in enumerate(pair):
                            off = (cj - 4 * gi) * 128 if cj > 4 * gi else 0
                            sh = off if (m == 1 and off > 0) else 0
                            offs.append(off)
                            shifts.append(sh)
                            nc.tensor.matmul(
                                pss[:, m * 512 + off - sh:
                                    (m + 1) * 512 - sh],
                                kT[:, cj * 128:(cj + 1) * 128],
                                qs[:, off:512],
                                start=True,
                                stop=True,
                            )
                        drain_filler(DRAIN[gi])
                        flush_pv(depth=PVDEPTH[gi])
                        pt = work.tile([128, 1024], BF16, tag="pt",
                                       name=f"pt{h}_{gi}_{p}")
                        nc.scalar.activation(
                            pt[:, offs[0]:1024 - shifts[1]],
                            pss[:, offs[0]:1024 - shifts[1]],
                            AF.Exp, scale=SCALE,
                        )
                        for m, cj in enumerate(pair):
                            v = cj - gi * 4
                            if v >= 0:
                                # multiplicative tril mask on the diagonal
                                lo = m * 512 + v * 128 - shifts[m]
                                nc.gpsimd.tensor_mul(
                                    pt[:, lo:lo + 128], pt[:, lo:lo + 128],
                                    tri_sb[:],
                                )

                        def mk_pv(po=po, pt=pt, pair=pair, gi=gi, h=h,
                                  state=state, shifts=tuple(shifts)):
                            def emit():
                                for m, cj in enumerate(pair):
                                    for icl in range(4):
                                        if 4 * gi + icl < cj:
                                            continue
                                        e = state["emitted"]
                                        lo = (m * 512 + icl * 128
                                              - shifts[m])
                                        nc.tensor.matmul(
                                            po[:, icl * 128:icl * 128 + DP],
                                            pt[:, lo:lo + 128],
                                            vs[cj][:, h, :],
                                            start=(e == 0),
                                            stop=(e == state["nblk"] - 1),
                                        )
                                        state["emitted"] = e + 1
                            return emit

                        pending_pv.append((mk_pv(), h // 2, pair))
                    return (h, gi, po)

                def emit_attn_tail(st, final=False):
                    """Normalize each query chunk's 64 columns by the
                    reciprocal of its ones-column sum; DVE reciprocal, then
                    per-partition scalar multiply straight out of PSUM. The
                    final segment splits the multiplies across ACT and DVE
                    (both idle by then) and ships each chunk's output slice
                    as its own mini-DMA so nothing serializes behind the
                    whole tail."""
                    h, gi, po = st
                    rec = work.tile([128, 4], F32, tag="rec",
                                    name=f"rec{h}_{gi}")
                    for icl in range(4):
                        nc.vector.reciprocal(
                            rec[:, icl:icl + 1],
                            po[:, icl * 128 + D:icl * 128 + DP],
                        )
                    for icl in range(4):
                        if final and icl % 2 == 0:
                            nc.scalar.activation(
                                outb[gi][:, icl, h * D:(h + 1) * D],
                                po[:, icl * 128:icl * 128 + D],
                                AF.Identity, scale=rec[:, icl:icl + 1],
                            )
                        else:
                            nc.vector.tensor_scalar_mul(
                                outb[gi][:, icl, h * D:(h + 1) * D],
                                po[:, icl * 128:icl * 128 + D],
                                rec[:, icl:icl + 1],
                            )
                    if final:
                        nc.sync.dma_start(
                            out_r[gi][:, :, h * D:NV],
                            outb[gi][:, :, h * D:NV],
                        )

                # emission order: gi outer / h inner so each output row block
                # completes early and DMAs out while compute continues.
                heads_done = [0] * TG
                tail = None
                for gi in range(TG):
                    for h in range(HPC):
                        st = emit_attn_main(h, gi)
                        if tail is not None:
                            emit_attn_tail(tail)
                            g_done = tail[1]
                            heads_done[g_done] += 1
                            if heads_done[g_done] == HPC:
                                nc.sync.dma_start(
                                    out_r[g_done], outb[g_done][:]
                                )
                            elif g_done == TG - 1 and heads_done[g_done] == 3:
                                # ship 3/4 of the last block early; only the
                                # final head's columns trail the last chain
                                nc.sync.dma_start(
                                    out_r[TG - 1][:, :, 0:3 * D],
                                    outb[TG - 1][:, :, 0:3 * D],
                                )
                        tail = st
                flush_pv()
                flush_filler()
                emit_attn_tail(tail, final=True)

    nc.compile()
    return nc


def make_tri():
    """Multiplicative causal mask for a 128x128 diagonal block of S^T[j, i]:
    1 where j <= i (attend), 0 where j > i (future). bf16."""
    import ml_dtypes

    jj = np.arange(128)[:, None]
    ii = np.arange(128)[None, :]
    return np.where(jj <= ii, 1.0, 0.0).astype(ml_dtypes.bfloat16)


def core_inputs(x, w_attn, b_attn, core):
    b = core // 4
    h0 = (core % 4) * HPC
    # n-chunk order [q01 | k01 | q23 | k23] (local head pairs)
    qk_cols = []
    for j in (0, 2):
        for base in (0, C):  # q chunk then k chunk for the pair
            for h in (h0 + j, h0 + j + 1):
                qk_cols.extend(range(base + h * D, base + (h + 1) * D))
    v_sl = slice(2 * C + h0 * D, 2 * C + (h0 + HPC) * D)
    import ml_dtypes

    bf16 = ml_dtypes.bfloat16
    # pre-shuffle weights to the SBUF tile layouts (contiguous DMAs):
    # w_qk[p, n, kc, c], w_v[p, kc*NV + c]
    wqk = w_attn[:, qk_cols].reshape(KC, 128, HPC, 128)
    wqk = np.ascontiguousarray(wqk.transpose(1, 2, 0, 3))
    wv = w_attn[:, v_sl].reshape(KC, 128, NV)
    wv = np.ascontiguousarray(wv.transpose(1, 0, 2)).reshape(128, KC * NV)
    return {
        "xT": np.ascontiguousarray(x[b].T).astype(bf16),
        "w_qk": wqk.astype(bf16),
        "w_v": wv.astype(bf16),
        "b_qk": np.ascontiguousarray(b_attn[qk_cols][:, None],
                                     dtype=np.float32),
        "b_v": np.ascontiguousarray(b_attn[v_sl][None, :], dtype=np.float32),
        "tri": make_tri(),
    }


_NC_CACHE = None


def run(x, w_attn, b_attn, **spmd_kwargs):
    """Run on the 8 NeuronCores; returns (full_output, BassKernelResults)."""
    global _NC_CACHE
    from concourse.bass_utils import run_bass_kernel_spmd

    x = np.asarray(x, dtype=np.float32)
    w_attn = np.asarray(w_attn, dtype=np.float32)
    b_attn = np.asarray(b_attn, dtype=np.float32)

    if _NC_CACHE is None:
        _NC_CACHE = build_nc()
    nc = _NC_CACHE

    in_maps = [core_inputs(x, w_attn, b_attn, c) for c in range(NCORES)]
    res = run_bass_kernel_spmd(
        nc, in_maps, core_ids=list(range(NCORES)), **spmd_kwargs
    )

    outf = np.empty((B, T, C), dtype=np.float32)
    for c in range(NCORES):
        b = c // 4
        h0 = (c % 4) * HPC
        outf[b, :, h0 * D:(h0 + HPC) * D] = res.results[c]["out"]
    return outf, res


def kernel(x, w_attn, b_attn):
    return run(x, w_attn, b_attn)[0]


# revision 80
# speedup vs baseline: 1.0764x; 1.0764x over previous
"""Multi-head causal self-attention on 8 Trainium2 NeuronCores.

Problem: x[2, 2048, 1024] @ w_attn[1024, 3072] (+b) -> split q,k,v (16 heads,
head_size 64) -> causal softmax attention -> out [2, 2048, 1024].

Sharding: 32 (batch, head) pairs across 8 cores -> each core handles one batch
and 4 consecutive heads (batch = core // 4, heads = (core % 4) * 4 + [0..3]).
Each core runs a fused QKV-projection + attention kernel on its slice; the
host assembles the full output. No collectives needed.

Per-core device kernel (the ACT exp stream is the co-bottleneck with the PE,
so the projection is dissolved into filler units dribbled through attention
instead of running as an ACT-idle prefix phase):
  - qkT layout is per HEAD PAIR: chunks [q01 | k01 | q23 | k23], each
    [128, T] bf16, so q_h and k_h share partition offset (h%2)*64 (w_qk
    columns host-reordered to match). Projection happens in 16 units
    U(n, qtr): 8 kc-stationary fp32r matmuls into a
    [128, 512] PSUM bank + one DVE bias-add (psum f32 -> bf16). V' is
    produced by 16 units V(jc) (8 matmuls + one DVE bias+pack to bf16 with
    a ones column). All 32 units live in one FIFO filler queue, popped by
    a baseline per-pair drain plus demand-driven flushes (QK of (h, gi)
    forces U(h, <=gi); PV of a pair forces its two V units). Head 0's
    attention starts after just U(0,0) (~2 us of PE), so the exp stream
    starts ~8 us in and the ACT engine never sits behind a projection
    phase.
  - Attention per (h, gi): S^T pair tiles [128, 1024] via kT-stationary
    bf16 matmuls, causally trimmed on the moving (query) dim; odd-slot
    diagonal chunks are column-shifted so each pair's live region is
    contiguous and ONE left-trimmed exp window covers it exactly -> pt
    bf16; tril mask on diagonal 128x128 sub-blocks (Pool). PV reoriented:
    out[i_chunk, d] accumulates in one PSUM bank ([128, 4x65] regions,
    one accumulation group per (h, gi): first write per address
    initializes, rest accumulate) with P^T blocks stationary and
    V'[128, 65] bf16 moving - only causal (ic >= cj) blocks stream,
    65 cycles each. No transpose matmuls anywhere.
  - Tail: DVE reciprocal of the ones-column sums, Pool per-partition
    scalar multiply straight out of PSUM into the output tiles, DMA per
    completed 4-chunk row block.

PSUM: psS 2x[128,1024] (4 banks) + po 2x[128,512] (2 banks) + filler
2x[128,512] (2 banks) = 8 banks exactly. The psS banks are DVE-zeroed once
up front so trimmed QK pairs leave only finite values in any never-written
columns an exp window reads. PV deferral depth must stay <= npairs(gi)-1
per group (deeper lets accumulation groups on the 2 rotating po banks
overlap and pt tiles outlive their buffer rotation).
"""

from collections import deque

import numpy as np

import concourse.bacc as bacc
import concourse.bass as bass
import concourse.tile as tile
from concourse import mybir

AF = mybir.ActivationFunctionType
F32 = mybir.dt.float32
F32R = mybir.dt.float32r
BF16 = mybir.dt.bfloat16

B, T, C = 2, 2048, 1024
H, D = 16, 64
HPC = 4                 # heads per core
NCORES = 8
KC = C // 128           # 8 contraction chunks
TC = T // 128           # 16 key/time chunks of 128
TG = T // 512           # 4 query groups of 512
NQK = 2 * HPC * D       # 512 q+k columns per core
NV = HPC * D            # 256 v columns per core
DP = D + 1              # v columns + ones column
SCALE = D ** -0.5
DRAIN = (4, 7, 3, 2)    # eager filler items per pair, by query group
PVDEPTH = (1, 3, 5, 7)  # deferred-PV queue depth, by query group


def build_nc(reps=1):
    """reps > 1 repeats the whole compute serially inside one NEFF -
    used only for differential on-device timing (tunnel overhead cancels)."""
    nc = bacc.Bacc("TRN2", target_bir_lowering=False, debug=False,
                   num_devices=NCORES)

    xT = nc.dram_tensor("xT", [C, T], BF16, kind="ExternalInput")
    # weights host-pre-shuffled to SBUF layout so their DMAs are contiguous
    # per partition (strided gathers would run at ~half DMA rate):
    # w_qk[p, n, kc, c] with n-chunk cols [q01|k01|q23|k23]; w_v[p, kc, c]
    w_qk = nc.dram_tensor("w_qk", [128, HPC, KC, 128], BF16,
                          kind="ExternalInput")
    w_v = nc.dram_tensor("w_v", [128, KC * NV], BF16, kind="ExternalInput")
    b_qk = nc.dram_tensor("b_qk", [NQK, 1], F32, kind="ExternalInput")
    b_v = nc.dram_tensor("b_v", [1, NV], F32, kind="ExternalInput")
    tri = nc.dram_tensor("tri", [128, 128], BF16, kind="ExternalInput")
    out = nc.dram_tensor("out", [T, NV], F32, kind="ExternalOutput")

    with tile.TileContext(nc) as tc:
      for rep in range(reps):
        with (
            tc.tile_pool(name=f"const{rep}", bufs=1) as cpool,
            tc.tile_pool(name=f"xw{rep}", bufs=1) as xw,
            tc.tile_pool(name=f"qkv{rep}", bufs=1) as qkv,
            tc.tile_pool(name=f"outp{rep}", bufs=1) as outp,
            tc.tile_pool(name=f"work{rep}", bufs=20) as work,
        ):
            # ---------- ACT exp-table warmup (hide the first-use load) ------
            warm = work.tile([128, 1], F32, tag="warm")
            nc.vector.memset(warm[:], 0.0)
            nc.scalar.activation(warm[:], warm[:], AF.Exp)

            # ---------- DMAs, in consumption order on the sync queue --------
            # U(0,0) runs first (needs wqk h0 + x quarter 0); V units need
            # wv/bv/tri by ~8us; wqk h1..h3 before x quarters 1..3 (their
            # units run later than (gi0, h1..h3)'s QK).
            # HWDGE issues ~1 DMACopy / 625ns, so coarse copies matter: one
            # strided copy per weight n-chunk and per x quarter. Quarter 0
            # stays per-kc so U(0,0)'s kc stream is paced by chunk arrival.
            wqk_sb = [xw.tile([128, KC, 128], BF16, tag=f"wqk{n}",
                              name=f"wqk{n}") for n in range(HPC)]
            xT_p = xT[:].rearrange("(kc p) t -> p kc t", p=128)
            # quarter 0 in four 2-chunk copies: few enough that the HWDGE
            # issue rate (~625ns/copy) isn't the startup bottleneck, small
            # enough that U(0/1,0)'s kc stream starts after the first copy
            xts0 = [xw.tile([128, 2, 512], BF16, tag=f"xt{kk}_0",
                            name=f"xt{kk}_0") for kk in range(4)]
            nc.sync.dma_start(wqk_sb[0][:], w_qk[:, 0])
            nc.sync.dma_start(wqk_sb[1][:], w_qk[:, 1])
            for kk in range(4):
                nc.sync.dma_start(
                    xts0[kk][:], xT_p[:, 2 * kk:2 * kk + 2, 0:512]
                )
            bqk_sb = cpool.tile([128, HPC, 1], F32)
            nc.sync.dma_start(
                bqk_sb[:], b_qk[:].rearrange("(c p) one -> p c one", p=128)
            )
            tri_sb = cpool.tile([128, 128], BF16)
            nc.sync.dma_start(tri_sb[:], tri[:])
            xq = [None] + [xw.tile([128, KC, 512], BF16, tag=f"xq{q}",
                                   name=f"xq{q}") for q in range(1, TG)]
            wv_sb = xw.tile([128, KC, NV], BF16)
            nc.sync.dma_start(
                wv_sb[:], w_v[:].rearrange("p (kc c) -> p kc c", kc=KC)
            )
            bv_sb = cpool.tile([128, NV], F32)
            nc.sync.dma_start(bv_sb[:], b_v[:].to_broadcast([128, NV]))
            for n in (2, 3):
                nc.sync.dma_start(wqk_sb[n][:], w_qk[:, n])
            for q in range(1, TG):
                nc.sync.dma_start(
                    xq[q][:], xT_p[:, :, q * 512:(q + 1) * 512]
                )

            def xslice(kc, q):
                return (xts0[kc // 2][:, kc % 2, :] if q == 0
                        else xq[q][:, kc, :])

            # ---------- persistent sbuf tiles ----------
            # n-chunk layout: [q01 | k01 | q23 | k23], so q_h and k_h sit at
            # the same partition offset (h%2)*64 of chunks (h//2)*2(+1) and
            # matmul operands share a base partition.
            qkts = [qkv.tile([128, T], BF16, tag=f"qk{n}", name=f"qk{n}")
                    for n in range(HPC)]
            # one tile per 512-row output block -> one strided DMA per block
            outb = [outp.tile([128, 4, NV], F32, tag=f"o{g}", name=f"o{g}")
                    for g in range(TG)]
            out_r = out[:].rearrange("(g ic p) c -> g p ic c", p=128, ic=4)

            with (
                tc.tile_pool(name=f"psS{rep}", bufs=2, space="PSUM") as psS,
                tc.tile_pool(name=f"psout{rep}", bufs=2, space="PSUM") as psout,
                tc.tile_pool(name=f"fps{rep}", bufs=2, space="PSUM") as fps,
            ):
                vs = [None] * TC
                v_done = [[False, False] for _ in range(TC)]  # per head-pair
                u_done = [[False] * TG for _ in range(HPC)]
                filler = deque()   # FIFO of (projection | v) unit items
                pending_pv = deque()  # (emit_fn, h, (cj, cj')) deferred pairs

                # p-state warm-up: ~4us of tiny matmuls under the initial DMA
                # wait so the first real matmuls run at full clock
                wt = work.tile([128, 16], F32, tag="wt")
                nc.vector.memset(wt[:], 0.0)
                pw = fps.tile([128, 512], F32, tag="f", name="pwarm")
                for i in range(220):
                    nc.tensor.matmul(pw[0:16, 0:4], wt[:, 0:16], wt[:, 0:4],
                                     start=True, stop=True)
                # zero both S-psum buffers once (DVE, while it idles under
                # the DMA wait) so trimmed QK pairs leave only finite values
                # in the dead columns the wide exp windows read
                for _ in range(2):
                    z = psS.tile([128, 1024], F32, tag="psS", name="zinit")
                    nc.vector.memset(z[:], 0.0)

                def queue_u2(na, nb, qtr, fini_act=False):
                    """qkts[na|nb][:, qtr] = (w_qk^T x^T-quarter) + bias for a
                    q/k chunk pair, kc-interleaved so both trail the x-chunk
                    DMA stream by one chunk instead of running back-to-back;
                    8+8 PE matmuls into two PSUM banks + 2 DVE bias-adds."""
                    pss = {n: fps.tile([128, 512], F32, tag="f",
                                       name=f"u{n}_{qtr}") for n in (na, nb)}

                    def mk_mm(n, kc):
                        def emit():
                            nc.tensor.matmul(
                                pss[n][:],
                                wqk_sb[n][:, kc, :],
                                xslice(kc, qtr),
                                start=(kc == 0),
                                stop=(kc == KC - 1),
                            )
                        return emit

                    def mk_fini(n):
                        def fini():
                            dst = qkts[n][:, qtr * 512:(qtr + 1) * 512]
                            if fini_act:
                                # startup-critical: the idle ACT engine
                                # finishes the first q/k tiles while DVE
                                # would still be queuing
                                nc.scalar.activation(
                                    dst, pss[n][:], AF.Identity,
                                    bias=bqk_sb[:, n, :], scale=1.0,
                                )
                            else:
                                nc.vector.tensor_scalar_add(
                                    dst, pss[n][:], bqk_sb[:, n, :],
                                )
                            u_done[n][qtr] = True
                        return fini

                    for kc in range(KC):
                        filler.append(mk_mm(na, kc))
                        filler.append(mk_mm(nb, kc))
                    filler.append(mk_fini(na))
                    filler.append(mk_fini(nb))

                def queue_v(jc, hp):
                    """vs[jc] head-pair hp = x^T[:, jc]^T @ w_v (+ bias, ones
                    column), packed bf16. Split per head pair so the h23
                    halves' deadlines land in the late (h2/h3) sweeps."""
                    if vs[jc] is None:
                        vs[jc] = qkv.tile([128, HPC, DP], BF16, tag=f"v{jc}",
                                          name=f"v{jc}")
                    vt = vs[jc]
                    ps = fps.tile([128, 512], F32, tag="f", name=f"v{jc}_{hp}")
                    cols = slice(hp * 128, (hp + 1) * 128)

                    def mk_mm(kc):
                        def emit():
                            xh = xslice(kc, jc // 4)
                            col = (jc % 4) * 128
                            nc.tensor.matmul(
                                ps[:, 0:128],
                                xh[:, col:col + 128],
                                wv_sb[:, kc, cols],
                                start=(kc == 0),
                                stop=(kc == KC - 1),
                            )
                        return emit

                    def fini():
                        # bias + pack 2 heads' v columns (bf16) in one op
                        nc.vector.tensor_add(
                            vt[:, 2 * hp:2 * hp + 2, 0:D], ps[:, 0:128],
                            bv_sb[:, cols],
                        )
                        nc.vector.memset(vt[:, 2 * hp:2 * hp + 2, D:DP], 1.0)
                        v_done[jc][hp] = True

                    for kc in range(KC):
                        filler.append(mk_mm(kc))
                    filler.append(fini)

                def drain_filler(n):
                    for _ in range(n):
                        if filler:
                            filler.popleft()()

                def flush_until(cond):
                    while filler and not cond():
                        filler.popleft()()

                def flush_filler():
                    while filler:
                        filler.popleft()()

                def flush_pv(depth=0):
                    """Emit deferred PV blocks until at most `depth` remain.
                    A deeper queue at the exp-heavy late groups lets the PE
                    run further ahead of the ACT stream."""
                    while len(pending_pv) > depth:
                        fn, hp, (c1, c2) = pending_pv.popleft()
                        flush_until(lambda: v_done[c1][hp] and v_done[c2][hp])
                        fn()

                # enqueue every unit up front, in deadline order. gi g needs
                # U(*, g) and V(4g..4g+3); diagonal-first pair order means
                # V(4g+2), V(4g+3) are consumed before V(4g), V(4g+1).
                # U23 sits before V(4g)/V(4g+1) so h0's second-pair PV pull
                # drags it through while ACT still has h0/h1 exp backlog,
                # instead of it bursting right before h2's first QK
                for g in range(TG):
                    queue_u2(0, 1, g, fini_act=(g == 0))
                    for jc in (4 * g + 2, 4 * g + 3, 4 * g, 4 * g + 1):
                        queue_v(jc, 0)
                    queue_u2(2, 3, g)
                    for jc in (4 * g + 2, 4 * g + 3, 4 * g, 4 * g + 1):
                        queue_v(jc, 1)

                def emit_attn_main(h, gi):
                    """QK -> exp -> (tril mask) -> PV, software-pipelined:
                    each pair's PV is emitted after the NEXT pair's QK so the
                    PE FIFO never heads-of-line-blocks on an exp. Diagonal
                    pairs first. Returns tail state."""
                    qn, kn = (h // 2) * 2, (h // 2) * 2 + 1
                    flush_until(lambda: all(u_done[n][q] for n in (qn, kn)
                                            for q in range(gi + 1)))
                    # cap cross-segment PV carryover: with depth[gi] <=
                    # npairs(gi), pendings at entry are from one segment
                    # back only, so po bank groups (2 rotating banks) never
                    # overlap
                    flush_pv(depth=2)
                    po_ = (h % 2) * D
                    qT = qkts[qn][po_:po_ + D, :]
                    kT = qkts[kn][po_:po_ + D, :]
                    qs = qT[:, gi * 512:(gi + 1) * 512]
                    ncj = gi * 4 + 4  # causal: j-chunks 0..gi*4+3
                    npair = ncj // 2
                    pair_order = list(range(npair - 1, -1, -1))  # diag first
                    # one PSUM bank holds all 4 query-chunk accumulators
                    # ([128, 65] regions at 128-col offsets) under a single
                    # accumulation group per (h, gi).
                    po = psout.tile([128, 512], F32, tag="po",
                                    name=f"po{h}_{gi}")
                    nblk = sum(1 for p in range(npair)
                               for cj in (2 * p, 2 * p + 1)
                               for icl in range(4) if 4 * gi + icl >= cj)
                    state = {"emitted": 0, "nblk": nblk}
                    for p in pair_order:
                        pair = (2 * p, 2 * p + 1)
                        pss = psS.tile([128, 1024], F32, tag="psS",
                                       name=f"pss{h}_{gi}_{p}")
                        # diagonal chunks in the odd slot are SHIFTED left so
                        # their live columns start at 512: each pair's live
                        # region is contiguous and one exp window covers it
                        # exactly (the psS banks were zeroed once up front,
                        # so any never-written column exps to a finite 1.0
                        # that PV never streams).
                        offs, shifts = [], []
                        for m, cj in enumerate(pair):
                            off = (cj - 4 * gi) * 128 if cj > 4 * gi else 0
                            sh = off if (m == 1 and off > 0) else 0
                            offs.append(off)
                            shifts.append(sh)
                            nc.tensor.matmul(
                                pss[:, m * 512 + off - sh:
                                    (m + 1) * 512 - sh],
                                kT[:, cj * 128:(cj + 1) * 128],
                                qs[:, off:512],
                                start=True,
                                stop=True,
                            )
                        drain_filler(DRAIN[gi])
                        flush_pv(depth=PVDEPTH[gi])
                        pt = work.tile([128, 1024], BF16, tag="pt",
                                       name=f"pt{h}_{gi}_{p}")
                        nc.scalar.activation(
                            pt[:, offs[0]:1024 - shifts[1]],
                            pss[:, offs[0]:1024 - shifts[1]],
                            AF.Exp, scale=SCALE,
                        )
                        for m, cj in enumerate(pair):
                            v = cj - gi * 4
                            if v >= 0:
                                # multiplicative tril mask on the diagonal
                                lo = m * 512 + v * 128 - shifts[m]
                                nc.gpsimd.tensor_mul(
                                    pt[:, lo:lo + 128], pt[:, lo:lo + 128],
                                    tri_sb[:],
                                )

                        def mk_pv(po=po, pt=pt, pair=pair, gi=gi, h=h,
                                  state=state, shifts=tuple(shifts)):
                            def emit():
                                for m, cj in enumerate(pair):
                                    for icl in range(4):
                                        if 4 * gi + icl < cj:
                                            continue
                                        e = state["emitted"]
                                        lo = (m * 512 + icl * 128
                                              - shifts[m])
                                        nc.tensor.matmul(
                                            po[:, icl * 128:icl * 128 + DP],
                                            pt[:, lo:lo + 128],
                                            vs[cj][:, h, :],
                                            start=(e == 0),
                                            stop=(e == state["nblk"] - 1),
                                        )
                                        state["emitted"] = e + 1
                            return emit

                        pending_pv.append((mk_pv(), h // 2, pair))
                    return (h, gi, po)

                def emit_attn_tail(st, final=False):
                    """Normalize each query chunk's 64 columns by the
                    reciprocal of its ones-column sum; DVE reciprocal, then
                    per-partition scalar multiply straight out of PSUM. The
                    final segment splits the multiplies across ACT and DVE
                    (both idle by then) and ships each chunk's output slice
                    as its own mini-DMA so nothing serializes behind the
                    whole tail."""
                    h, gi, po = st
                    rec = work.tile([128, 4], F32, tag="rec",
                                    name=f"rec{h}_{gi}")
                    for icl in range(4):
                        nc.vector.reciprocal(
                            rec[:, icl:icl + 1],
                            po[:, icl * 128 + D:icl * 128 + DP],
                        )
                    for icl in range(4):
                        if final and icl % 2 == 0:
                            nc.scalar.activation(
                                outb[gi][:, icl, h * D:(h + 1) * D],
                                po[:, icl * 128:icl * 128 + D],
                                AF.Identity, scale=rec[:, icl:icl + 1],
                            )
                        else:
                            nc.vector.tensor_scalar_mul(
                                outb[gi][:, icl, h * D:(h + 1) * D],
                                po[:, icl * 128:icl * 128 + D],
                                rec[:, icl:icl + 1],
                            )
                    if final:
                        nc.sync.dma_start(
                            out_r[gi][:, :, h * D:NV],
                            outb[gi][:, :, h * D:NV],
                        )

                # emission order: gi outer / h inner so each output row block
                # completes early and DMAs out while compute continues.
                heads_done = [0] * TG
                tail = None
                for gi in range(TG):
                    for h in range(HPC):
                        st = emit_attn_main(h, gi)
                        if tail is not None:
                            emit_attn_tail(tail)
                            g_done = tail[1]
                            heads_done[g_done] += 1
                            if heads_done[g_done] == HPC:
                                nc.sync.dma_start(
                                    out_r[g_done], outb[g_done][:]
                                )
                            elif g_done == TG - 1 and heads_done[g_done] == 3:
                                # ship 3/4 of the last block early; only the
                                # final head's columns trail the last chain
                                nc.sync.dma_start(
                                    out_r[TG - 1][:, :, 0:3 * D],
                                    outb[TG - 1][:, :, 0:3 * D],
                                )
                        tail = st
                flush_pv()
                flush_filler()
                emit_attn_tail(tail, final=True)

    nc.compile()
    return nc


def make_tri():
    """Multiplicative causal mask for a 128x128 diagonal block of S^T[j, i]:
    1 where j <= i (attend), 0 where j > i (future). bf16."""
    import ml_dtypes

    jj = np.arange(128)[:, None]
    ii = np.arange(128)[None, :]
    return np.where(jj <= ii, 1.0, 0.0).astype(ml_dtypes.bfloat16)


def core_inputs(x, w_attn, b_attn, core):
    b = core // 4
    h0 = (core % 4) * HPC
    # n-chunk order [q01 | k01 | q23 | k23] (local head pairs)
    qk_cols = []
    for j in (0, 2):
        for base in (0, C):  # q chunk then k chunk for the pair
            for h in (h0 + j, h0 + j + 1):
                qk_cols.extend(range(base + h * D, base + (h + 1) * D))
    v_sl = slice(2 * C + h0 * D, 2 * C + (h0 + HPC) * D)
    import ml_dtypes

    bf16 = ml_dtypes.bfloat16
    # pre-shuffle weights to the SBUF tile layouts (contiguous DMAs):
    # w_qk[p, n, kc, c], w_v[p, kc*NV + c]
    wqk = w_attn[:, qk_cols].reshape(KC, 128, HPC, 128)
    wqk = np.ascontiguousarray(wqk.transpose(1, 2, 0, 3))
    wv = w_attn[:, v_sl].reshape(KC, 128, NV)
    wv = np.ascontiguousarray(wv.transpose(1, 0, 2)).reshape(128, KC * NV)
    return {
        "xT": np.ascontiguousarray(x[b].T).astype(bf16),
        "w_qk": wqk.astype(bf16),
        "w_v": wv.astype(bf16),
        "b_qk": np.ascontiguousarray(b_attn[qk_cols][:, None],
                                     dtype=np.float32),
        "b_v": np.ascontiguousarray(b_attn[v_sl][None, :], dtype=np.float32),
        "tri": make_tri(),
    }


_NC_CACHE = None


def run(x, w_attn, b_attn, **spmd_kwargs):
    """Run on the 8 NeuronCores; returns (full_output, BassKernelResults)."""
    global _NC_CACHE
    from concourse.bass_utils import run_bass_kernel_spmd

    x = np.asarray(x, dtype=np.float32)
    w_attn = np.asarray(w_attn, dtype=np.float32)
    b_attn = np.asarray(b_attn, dtype=np.float32)

    if _NC_CACHE is None:
        _NC_CACHE = build_nc()
    nc = _NC_CACHE

    in_maps = [core_inputs(x, w_attn, b_attn, c) for c in range(NCORES)]
    res = run_bass_kernel_spmd(
        nc, in_maps, core_ids=list(range(NCORES)), **spmd_kwargs
    )

    outf = np.empty((B, T, C), dtype=np.float32)
    for c in range(NCORES):
        b = c // 4
        h0 = (c % 4) * HPC
        outf[b, :, h0 * D:(h0 + HPC) * D] = res.results[c]["out"]
    return outf, res


def kernel(x, w_attn, b_attn):
    return run(x, w_attn, b_attn)[0]
